# revision 1
# baseline (speedup 1.0000x reference)
"""Trainium2 Bass kernel for nn_BiAttention (MoE-routed bi-attention).

Strategy (8 NeuronCores, SPMD single program):
- Data-parallel over batch: core c handles batches [4c, 4c+4).
- Expert routing handled on host as part of sharding: within each batch the
  512 tokens are stable-sorted by expert, and each (batch, expert) segment is
  zero-padded to a GLOBAL per-expert capacity C_e (max over all 32 batches),
  so the device program is identical on every core. Per-core routing lives
  only in the data: the padded/permuted x^T inputs and a key-mask bias tensor.
- Projections: out^T = W^T @ x (tokens on the moving free dim), bf16 inputs,
  fp32 PSUM accumulation over the 4 contraction slabs of d=512.
- Attention is computed in "scores-transposed" layout: scoresT[k,q] = K·Q per
  head; exp via ScalarE with per-partition mask bias (-1e4 on padded keys);
  P·V via matmul with V rows augmented by a ones column, which yields the
  softmax denominator Z as row 64 of the output for free; normalization via
  reciprocal + ones-matmul broadcast + elementwise multiply.
- Outputs are produced transposed ([h*dk, token]); the host transposes and
  un-permutes during unshard.
"""
import math

import numpy as np
import ml_dtypes

import concourse.bass as bass
import concourse.mybir as mybir
import concourse.tile as tile
from concourse.bass_utils import run_bass_kernel_spmd

F32 = mybir.dt.float32
BF16 = mybir.dt.bfloat16

B, N, D, H, DK, NE = 32, 512, 512, 8, 64, 9
NCORES = 8
BL = B // NCORES  # batches per core

ENGINE_OK = {
    mybir.EngineType.PE,
    mybir.EngineType.Activation,
    mybir.EngineType.DVE,
    mybir.EngineType.Pool,
    mybir.EngineType.SP,
}


def _fix_multiwait(nc, cap_default=1, cap_evsem=2):
    """walrus in this container accepts at most 1 sync-wait per instruction;
    move excess waits onto freshly inserted same-engine NoOps."""
    uid = 0
    for fn in nc.m.functions:
        for bb in fn.blocks:
            insts = bb.instructions
            i = 0
            while i < len(insts):
                ins = insts[i]
                si = getattr(ins, "sync_info", None)
                waits = list(si.on_wait) if (si and si.on_wait) else []
                cap = cap_evsem if isinstance(ins, mybir.InstEventSemaphore) else cap_default
                if len(waits) > cap and ins.engine in ENGINE_OK:
                    extra, keep = waits[:-cap], waits[-cap:]
                    si.on_wait = keep
                    nops = []
                    for w in extra:
                        uid += 1
                        nops.append(mybir.InstNoOp(
                            name=f"I-mwfix-{uid}",
                            engine=ins.engine,
                            ins=[], outs=[],
                            sync_info=mybir.SyncInfo(on_wait=[w], on_update=[]),
                            text_hint="multiwait_fix",
                        ))
                    insts[i:i] = nops
                    i += len(nops)
                i += 1


def _build_program(caps, starts, L):
    """Build the SPMD bass program. caps/starts: per-expert capacity and start
    column inside each batch's padded region of length L."""
    KS = -(-L // 128)
    ks_sizes = [128] * (KS - 1) + [L - 128 * (KS - 1)]
    LBL = BL * L

    nc = bass.Bass()
    x_d = [nc.dram_tensor("x1", [D, LBL], BF16, kind="ExternalInput"),
           nc.dram_tensor("x2", [D, LBL], BF16, kind="ExternalInput")]
    w_d = [nc.dram_tensor("wi", [3, NE, D, D], BF16, kind="ExternalInput"),
           nc.dram_tensor("wt", [3, NE, D, D], BF16, kind="ExternalInput")]
    bias_d = nc.dram_tensor("bias", [BL, KS, 128], F32, kind="ExternalInput")
    id_d = nc.dram_tensor("iden", [128, 128], BF16, kind="ExternalInput")
    o_d = [nc.dram_tensor("o1", [BL, D, L], F32, kind="ExternalOutput"),
           nc.dram_tensor("o2", [BL, D, L], F32, kind="ExternalOutput")]

    with tile.TileContext(nc) as tc:
        with (
            tc.tile_pool(name="const", bufs=1) as constp,
            tc.tile_pool(name="qk", bufs=1) as qkp,
            tc.tile_pool(name="vsb", bufs=1) as vp,
        ):
            id_sb = constp.tile([128, 128], BF16)
            nc.sync.dma_start(id_sb[:], id_d[:])
            ones_sb = constp.tile([1, DK], BF16)
            nc.vector.memset(ones_sb[:], 1.0)
            bias_sb = constp.tile([128, BL, KS], F32)
            nc.sync.dma_start(bias_sb[:], bias_d.rearrange("b k p -> p b k"))

            # persistent Q^T/K^T tiles per side, and V (token-major) per side
            qt = [qkp.tile([128, 4, LBL], BF16, tag=f"qt{s}", name=f"qt{s}")
                  for s in range(2)]
            kt = [qkp.tile([128, 4, LBL], BF16, tag=f"kt{s}", name=f"kt{s}")
                  for s in range(2)]
            v_sb = [vp.tile([128, BL, KS, H, DK + 1], BF16, tag=f"v{s}", name=f"v{s}")
                    for s in range(2)]
            nc.vector.memset(v_sb[0][:], 1.0)
            nc.vector.memset(v_sb[1][:], 1.0)

            # ---- Phase P: projections (+ phase T: V transposes per side) ----
            with (
                tc.tile_pool(name="xp", bufs=1) as xp,
                tc.tile_pool(name="vt", bufs=1) as vtp,
                tc.tile_pool(name="wp", bufs=2) as wp,
                tc.tile_pool(name="pp", bufs=4, space="PSUM") as ppool,
                tc.tile_pool(name="tp", bufs=4, space="PSUM") as tpool,
            ):
                copy_i = 0
                for si in range(2):
                    x_sb = xp.tile([128, 4, LBL], BF16, tag="x")
                    nc.sync.dma_start(
                        x_sb[:], x_d[si].rearrange("(ks p) t -> p ks t", p=128))
                    vt_sb = vtp.tile([128, 4, LBL], BF16, tag="vt")
                    dsts = [qt[si], kt[si], vt_sb]
                    for i in range(3):
                        dst = dsts[i]
                        for e in range(NE):
                            ce = caps[e]
                            if ce == 0:
                                continue
                            w_sb = wp.tile([128, 4, D], BF16, tag="w")
                            nc.sync.dma_start(
                                w_sb[:],
                                w_d[si][i, e].rearrange("(ks p) o -> p ks o", p=128))
                            xe = (x_sb[:, :, :]
                                  .rearrange("p ks (b l) -> p ks b l", b=BL)
                                  [:, :, :, starts[e]:starts[e] + ce])
                            for ms in range(4):
                                pp = ppool.tile([128, BL, ce], F32, tag="pp")
                                for ksl in range(4):
                                    nc.tensor.matmul(
                                        pp[:, :, :],
                                        w_sb[:, ksl, ms * 128:(ms + 1) * 128],
                                        xe[:, ksl, :, :],
                                        start=(ksl == 0),
                                        stop=(ksl == 3),
                                    )
                                de = (dst[:, ms, :]
                                      .rearrange("p (b l) -> p b l", b=BL)
                                      [:, :, starts[e]:starts[e] + ce])
                                if copy_i % 2 == 0:
                                    nc.scalar.copy(de, pp[:, :, :])
                                else:
                                    nc.vector.tensor_copy(de, pp[:, :, :])
                                copy_i += 1
                    # V transposes for this side: vt (dim-major) -> v_sb rows
                    for b in range(BL):
                        for ms in range(4):
                            for ksl in range(KS):
                                sz = ks_sizes[ksl]
                                tv = tpool.tile([128, 128], BF16, tag="tv")
                                nc.tensor.transpose(
                                    tv[0:sz, :],
                                    vt_sb[:, ms, b * L + ksl * 128:
                                          b * L + ksl * 128 + sz],
                                    id_sb[:],
                                )
                                src = tv[0:sz, :].rearrange(
                                    "k (h2 dd) -> k h2 dd", h2=2)
                                de = v_sb[si][0:sz, b, ksl,
                                              2 * ms:2 * ms + 2, 0:DK]
                                if copy_i % 2 == 0:
                                    nc.scalar.copy(de, src)
                                else:
                                    nc.vector.tensor_copy(de, src)
                                copy_i += 1

            # ---- Phase A: attention ----
            with (
                tc.tile_pool(name="ep", bufs=8) as ep,
                tc.tile_pool(name="op", bufs=3) as op_,
                tc.tile_pool(name="sp", bufs=2, space="PSUM") as sp,
                tc.tile_pool(name="oo", bufs=1, space="PSUM") as oop,
                tc.tile_pool(name="rp", bufs=1, space="PSUM") as rpp,
            ):
                scale = 1.0 / math.sqrt(DK)
                for b in range(BL):
                    for att in range(2):
                        qs = 1 - att   # h1: Q from type side (x2); h2: from item
                        kvs = att
                        for h in range(H):
                            ms, poff = h // 2, (h % 2) * 64
                            po = oop.tile([128, L], F32, tag="po")
                            qch = [(0, min(512, L))]
                            if L > 512:
                                qch.append((512, L - 512))
                            es = []
                            for ksl in range(KS):
                                sz = ks_sizes[ksl]
                                ps = sp.tile([128, L], F32, tag="ps")
                                for (q0, qn) in qch:
                                    nc.tensor.matmul(
                                        ps[0:sz, q0:q0 + qn],
                                        kt[kvs][poff:poff + 64, ms,
                                                b * L + ksl * 128:
                                                b * L + ksl * 128 + sz],
                                        qt[qs][poff:poff + 64, ms,
                                               b * L + q0:b * L + q0 + qn],
                                        start=True, stop=True,
                                    )
                                e_sb = ep.tile([128, L], BF16, tag="E")
                                nc.scalar.activation(
                                    e_sb[0:sz, :], ps[0:sz, :],
                                    mybir.ActivationFunctionType.Exp,
                                    bias=bias_sb[0:sz, b, ksl:ksl + 1],
                                    scale=scale,
                                )
                                es.append(e_sb)
                            for ksl in range(KS):
                                sz = ks_sizes[ksl]
                                for (q0, qn) in qch:
                                    nc.tensor.matmul(
                                        po[0:DK + 1, q0:q0 + qn],
                                        v_sb[kvs][0:sz, b, ksl, h, :],
                                        es[ksl][0:sz, q0:q0 + qn],
                                        start=(ksl == 0),
                                        stop=(ksl == KS - 1),
                                    )
                            recip = op_.tile([1, L], F32, tag="rc")
                            nc.vector.reciprocal(recip[:], po[DK:DK + 1, :])
                            rcb = op_.tile([1, L], BF16, tag="rcb")
                            nc.vector.tensor_copy(rcb[:], recip[:])
                            rb = rpp.tile([DK, L], F32, tag="rb")
                            for (q0, qn) in qch:
                                nc.tensor.matmul(rb[:, q0:q0 + qn], ones_sb[:],
                                                 rcb[:, q0:q0 + qn],
                                                 start=True, stop=True)
                            o_tmp = op_.tile([DK, L], F32, tag="ot")
                            nc.vector.tensor_copy(o_tmp[:], po[0:DK, :])
                            o_sb = op_.tile([DK, L], F32, tag="o")
                            nc.vector.tensor_tensor(
                                out=o_sb[:], in0=o_tmp[:], in1=rb[:],
                                op=mybir.AluOpType.mult,
                            )
                            nc.sync.dma_start(
                                o_d[att][b, h * 64:(h + 1) * 64, :], o_sb[:])

    _fix_multiwait(nc)
    return nc


def kernel(hidden1, hidden2, mask, b_seq, W_item, W_type):
    hidden1 = np.asarray(hidden1, dtype=np.float32)
    hidden2 = np.asarray(hidden2, dtype=np.float32)
    mask = np.asarray(mask).astype(bool)
    b_seq = np.asarray(b_seq, dtype=np.int32)
    W_item = np.asarray(W_item, dtype=np.float32)
    W_type = np.asarray(W_type, dtype=np.float32)

    # --- routing metadata (host): per-batch stable sort by expert ---
    cnt = np.zeros((B, NE), dtype=np.int64)
    for e in range(NE):
        cnt[:, e] = (b_seq == e).sum(axis=1)
    caps = cnt.max(axis=0).astype(int)          # global per-expert capacity
    starts = np.concatenate([[0], np.cumsum(caps)[:-1]]).astype(int)
    L = int(caps.sum())
    KS = -(-L // 128)

    # column position of each token inside its batch's padded region
    colmap = np.zeros((B, N), dtype=np.int64)
    for b in range(B):
        off = np.zeros(NE, dtype=np.int64)
        for n in range(N):
            e = b_seq[b, n]
            colmap[b, n] = starts[e] + off[e]
            off[e] += 1

    # --- per-core inputs ---
    wi = W_item.reshape(3, NE, D, H * DK).astype(ml_dtypes.bfloat16)
    wt = W_type.reshape(3, NE, D, H * DK).astype(ml_dtypes.bfloat16)
    iden = np.eye(128, dtype=np.float32).astype(ml_dtypes.bfloat16)

    in_maps = []
    for c in range(NCORES):
        x1 = np.zeros((D, BL * L), dtype=np.float32)
        x2 = np.zeros((D, BL * L), dtype=np.float32)
        biasp = np.full((BL, KS * 128), -10000.0, dtype=np.float32)
        for bl in range(BL):
            g = c * BL + bl
            cols = bl * L + colmap[g]
            x1[:, cols] = hidden1[g].T
            x2[:, cols] = hidden2[g].T
            # real tokens get bias 0; padded slots and masked-out keys keep
            # -1e4 so exp() zeroes them (matches reference's -1e30 masking)
            biasp[bl, colmap[g][mask[g]]] = 0.0
        in_maps.append({
            "x1": x1.astype(ml_dtypes.bfloat16),
            "x2": x2.astype(ml_dtypes.bfloat16),
            "wi": wi, "wt": wt,
            "bias": biasp.reshape(BL, KS, 128),
            "iden": iden,
        })

    nc = _build_program(caps, starts, L)
    res = run_bass_kernel_spmd(nc, in_maps, list(range(NCORES)))

    # --- unshard: transpose + un-permute ---
    h1 = np.zeros((B, N, D), dtype=np.float32)
    h2 = np.zeros((B, N, D), dtype=np.float32)
    for c in range(NCORES):
        o1 = res.results[c]["o1"]
        o2 = res.results[c]["o2"]
        for bl in range(BL):
            g = c * BL + bl
            h1[g] = o1[bl][:, colmap[g]].T
            h2[g] = o2[bl][:, colmap[g]].T
    return (h1, h2)

